# revision 18
# baseline (speedup 1.0000x reference)
import sys

sys.path.insert(0, "/opt/trn_rl_repo")
import numpy as np
import ml_dtypes

import concourse.bass as bass
import concourse.tile as tile
import concourse.bacc as bacc
from concourse import mybir
from concourse.bass_utils import run_bass_kernel_spmd

# bass_utils' axon trace path hard-imports antenv.axon_hooks; provide a
# null-hook shim when the image lacks it so tracing degrades gracefully
# instead of crashing kernel().
try:
    import antenv.axon_hooks  # noqa: F401
except ImportError:
    import types as _types

    _hook_store = {"fn": None}
    _m = _types.ModuleType("antenv.axon_hooks")
    _m.set_axon_ntff_profile_hook = lambda h: _hook_store.__setitem__("fn", h)
    _m.get_axon_ntff_profile_hook = lambda: _hook_store["fn"]
    sys.modules["antenv.axon_hooks"] = _m

BF16 = mybir.dt.bfloat16
F32 = mybir.dt.float32
FP8 = mybir.dt.float8e4
DR = mybir.MatmulPerfMode.DoubleRow
RELU = mybir.ActivationFunctionType.Relu

N_CORES = 8
EMBED = 768
KC = 6            # 768 / 128 contraction chunks
BLOCKS = 8
BS = 96
LATENT = 4 * EMBED            # 3072
HID_M = 4 * LATENT            # 12288
OUT_M = 2 * LATENT            # 6144
HID_F = 4 * EMBED             # 3072
OUT_F = 2 * EMBED             # 1536
LAMBD = 0.01
EPS = 1e-5
H = 128
W = 128
WF = 65
SPEC_TOT = H * WF             # 8320
S1 = (H * W) // N_CORES       # 2048 spatial px per core
S2 = SPEC_TOT // N_CORES      # 1040 spectral px per core
PXF = 2 * S2                  # 2080 (re ++ im)
NBF = 5
BLKF = PXF // NBF             # 416 (psum-bank safe)


def _erf(x):
    a1, a2, a3, a4, a5, p = (
        0.254829592, -0.284496736, 1.421413741, -1.453152027, 1.061405429, 0.3275911,
    )
    s = np.sign(x)
    ax = np.abs(x)
    t = 1.0 / (1.0 + p * ax)
    y = 1.0 - (((((a5 * t + a4) * t) + a3) * t + a2) * t + a1) * t * np.exp(-ax * ax)
    return s * y


def _gelu(x):
    return 0.5 * x * (1.0 + _erf(x / np.sqrt(2.0)))


def _layernorm(x, w, b):
    m = x.mean(-1, keepdims=True)
    v = x.var(-1, keepdims=True)
    return (x - m) / np.sqrt(v + EPS) * w + b


def _softshrink(x, l):
    return np.where(x > l, x - l, np.where(x < -l, x + l, 0.0)).astype(np.float32)


def _blockmm(x, w):
    return np.einsum("nyxbi,bio->nyxbo", x, w, optimize=True)


_PROGRAM = None
LAST_RESULT = None


def _build_program():
    global _PROGRAM
    if _PROGRAM is not None:
        return _PROGRAM
    from contextlib import ExitStack

    nc = bacc.Bacc("TRN2", target_bir_lowering=False, debug=False, num_devices=N_CORES)

    A1 = nc.dram_tensor("a1", [128, KC, S1], FP8, kind="ExternalInput")
    W1M = nc.dram_tensor("w1m", [128, HID_M // 128, KC, 128], FP8, kind="ExternalInput")
    W2M = nc.dram_tensor("w2m", [OUT_M, HID_M // 128, 128], FP8, kind="ExternalInput")
    B1M = nc.dram_tensor("b1m", [128, HID_M // 128], F32, kind="ExternalInput")
    B2M = nc.dram_tensor("b2m", [128, OUT_M // 128], F32, kind="ExternalInput")
    A23 = nc.dram_tensor("a23", [128, KC, PXF], FP8, kind="ExternalInput")
    W1F = nc.dram_tensor("w1f", [128, HID_F // 128, KC, 128], FP8, kind="ExternalInput")
    W2F8 = nc.dram_tensor("w2f8", [OUT_F // 2, HID_F // 128, 128], FP8, kind="ExternalInput")
    W2FB = nc.dram_tensor("w2fb", [OUT_F // 2, HID_F // 128, 128], BF16, kind="ExternalInput")
    B1F = nc.dram_tensor("b1f", [128, HID_F // 128], F32, kind="ExternalInput")
    B2F = nc.dram_tensor("b2f", [128, OUT_F // 128], F32, kind="ExternalInput")

    O1 = nc.dram_tensor("o1", [OUT_M, S1], BF16, kind="ExternalOutput")
    O2 = nc.dram_tensor("o2", [OUT_F, PXF], BF16, kind="ExternalOutput")

    HM = HID_M // 128   # 96
    OM = OUT_M // 128   # 48
    HF = HID_F // 128   # 24
    OF = OUT_F // 128   # 12

    with tile.TileContext(nc) as tc, ExitStack() as octx:
        # ---------- M pipeline: fp8 DoubleRow, 2 pixel halves of 1024 ----------
        with ExitStack() as mctx:
            cp = mctx.enter_context(tc.tile_pool(name="m_const", bufs=1))
            w1p = mctx.enter_context(tc.tile_pool(name="m_w1", bufs=2))
            w2p = mctx.enter_context(tc.tile_pool(name="m_w2", bufs=2))
            h1p = mctx.enter_context(tc.tile_pool(name="m_h1", bufs=1))
            op = mctx.enter_context(tc.tile_pool(name="m_out", bufs=4))
            pp = mctx.enter_context(tc.tile_pool(name="m_ps", bufs=8, space="PSUM"))

            a1t = cp.tile([128, KC, S1], FP8)
            nc.sync.dma_start(a1t[:], A1[:])
            b1t = cp.tile([128, HM], F32)
            nc.sync.dma_start(b1t[:], B1M[:])
            b2t = cp.tile([128, OM], F32)
            nc.sync.dma_start(b2t[:], B2M[:])

            for hf in range(2):
                h1t = h1p.tile([128, HM, 1024], FP8, tag="h1")
                # conv1: 96 hid strips in groups of 8
                for g in range(12):
                    w1t = w1p.tile([128, 8, KC, 128], FP8, tag="w1")
                    nc.sync.dma_start(w1t[:], W1M[:, bass.ds(g * 8, 8), :, :])
                    for s in range(8):
                        i = g * 8 + s
                        for sb in range(2):
                            ps = pp.tile([128, 512], F32, tag="ps")
                            for c in range(3):
                                nc.tensor.matmul(
                                    ps[:],
                                    w1t[:, s, bass.ds(2 * c, 2), :],
                                    a1t[:, bass.ds(2 * c, 2), bass.ds(hf * 1024 + sb * 512, 512)],
                                    start=(c == 0), stop=(c == 2),
                                    perf_mode=DR,
                                )
                            nc.scalar.activation(
                                h1t[:, i, bass.ds(sb * 512, 512)], ps[:], RELU,
                                bias=b1t[:, i:i + 1],
                            )
                # conv2: 48 out strips, stream weights
                for o in range(OM):
                    w2t = w2p.tile([128, HM, 128], FP8, tag="w2")
                    nc.sync.dma_start(w2t[:], W2M[bass.ds(o * 128, 128), :, :])
                    for sb in range(2):
                        ps = pp.tile([128, 512], F32, tag="ps")
                        for j in range(48):
                            nc.tensor.matmul(
                                ps[:],
                                w2t[:, bass.ds(2 * j, 2), :],
                                h1t[:, bass.ds(2 * j, 2), bass.ds(sb * 512, 512)],
                                start=(j == 0), stop=(j == 47),
                                perf_mode=DR,
                            )
                        ot = op.tile([128, 512], BF16, tag="ot")
                        nc.scalar.activation(ot[:], ps[:], RELU, bias=b2t[:, o:o + 1])
                        nc.sync.dma_start(
                            O1[bass.ds(o * 128, 128), bass.ds(hf * 1024 + sb * 512, 512)],
                            ot[:],
                        )

        # ---------- F pipeline: fp8 conv1 (DoubleRow) + bf16 conv2 ----------
        with ExitStack() as fctx:
            cfp = fctx.enter_context(tc.tile_pool(name="f_const", bufs=1))
            w2fp = fctx.enter_context(tc.tile_pool(name="f_w2", bufs=2))
            h1fp = fctx.enter_context(tc.tile_pool(name="f_h1", bufs=1))
            ofp = fctx.enter_context(tc.tile_pool(name="f_out", bufs=4))
            fpp = fctx.enter_context(tc.tile_pool(name="f_ps", bufs=8, space="PSUM"))

            a23t = cfp.tile([128, KC, PXF], FP8)
            nc.sync.dma_start(a23t[:], A23[:])
            w1ft = cfp.tile([128, HF, KC, 128], FP8)
            nc.sync.dma_start(w1ft[:], W1F[:])
            fb1t = cfp.tile([128, HF], F32)
            nc.sync.dma_start(fb1t[:], B1F[:])
            fb2t = cfp.tile([128, OF], F32)
            nc.sync.dma_start(fb2t[:], B2F[:])

            h1ft = h1fp.tile([128, HF, PXF], BF16)
            h1f8t = h1fp.tile([128, HF, PXF], FP8)
            for i in range(HF):
                for nb in range(NBF):
                    ps = fpp.tile([128, BLKF], F32, tag="ps")
                    for c in range(KC // 2):
                        nc.tensor.matmul(
                            ps[:],
                            w1ft[:, i, bass.ds(2 * c, 2), :],
                            a23t[:, bass.ds(2 * c, 2), bass.ds(nb * BLKF, BLKF)],
                            start=(c == 0), stop=(c == KC // 2 - 1),
                            perf_mode=DR,
                        )
                    nc.scalar.activation(
                        h1ft[:, i, bass.ds(nb * BLKF, BLKF)], ps[:], RELU,
                        bias=fb1t[:, i:i + 1],
                    )
                    nc.vector.tensor_copy(
                        h1f8t[:, i, bass.ds(nb * BLKF, BLKF)],
                        h1ft[:, i, bass.ds(nb * BLKF, BLKF)],
                    )
            # scale half (output rows 0:768): fp8 DoubleRow — the scale
            # multiplies the small-amplitude spectral signal, so its fp8
            # noise is strongly attenuated; shift half stays bf16.
            for o in range(OF // 2):
                w2ft = w2fp.tile([128, HF, 128], FP8, tag="w2f8")
                nc.sync.dma_start(w2ft[:], W2F8[bass.ds(o * 128, 128), :, :])
                for nb in range(NBF):
                    ps = fpp.tile([128, BLKF], F32, tag="ps")
                    for j in range(HF // 2):
                        nc.tensor.matmul(
                            ps[:],
                            w2ft[:, bass.ds(2 * j, 2), :],
                            h1f8t[:, bass.ds(2 * j, 2), bass.ds(nb * BLKF, BLKF)],
                            start=(j == 0), stop=(j == HF // 2 - 1),
                            perf_mode=DR,
                        )
                    ot = ofp.tile([128, BLKF], BF16, tag="otf")
                    nc.scalar.activation(ot[:], ps[:], RELU, bias=fb2t[:, o:o + 1])
                    nc.sync.dma_start(
                        O2[bass.ds(o * 128, 128), bass.ds(nb * BLKF, BLKF)], ot[:]
                    )
            for oo in range(OF // 2):
                o = OF // 2 + oo
                w2ft = w2fp.tile([128, HF, 128], BF16, tag="w2fb")
                nc.sync.dma_start(w2ft[:], W2FB[bass.ds(oo * 128, 128), :, :])
                for nb in range(NBF):
                    ps = fpp.tile([128, BLKF], F32, tag="ps")
                    for j in range(HF):
                        nc.tensor.matmul(
                            ps[:],
                            w2ft[:, j, :],
                            h1ft[:, j, bass.ds(nb * BLKF, BLKF)],
                            start=(j == 0), stop=(j == HF - 1),
                        )
                    ot = ofp.tile([128, BLKF], BF16, tag="otf")
                    nc.scalar.activation(ot[:], ps[:], RELU, bias=fb2t[:, o:o + 1])
                    nc.sync.dma_start(
                        O2[bass.ds(o * 128, 128), bass.ds(nb * BLKF, BLKF)], ot[:]
                    )

    nc.compile()
    _PROGRAM = nc
    return nc


def _fp8(x):
    return np.clip(np.ascontiguousarray(x), -240, 240).astype(ml_dtypes.float8_e4m3)


def _bf16(x):
    return np.ascontiguousarray(x).astype(ml_dtypes.bfloat16)


def kernel(x, mod_embed, norm1_w, norm1_b, norm2_w, norm2_b, w1, b1, w2, b2,
           f_c1_w, f_c1_b, f_c2_w, f_c2_b, fc1_w, fc1_b, fc2_w, fc2_b,
           m_c1_w, m_c1_b, m_c2_w, m_c2_b):
    x = np.asarray(x, np.float32)
    mod_embed = np.asarray(mod_embed, np.float32)
    B = x.shape[0]
    assert B == 1 and x.shape == (1, H, W, EMBED)

    # ---- host: LN1 + forward FFTs (cheap) ----
    residual = x
    xn = _layernorm(x, np.asarray(norm1_w, np.float32), np.asarray(norm1_b, np.float32))
    xf = np.fft.rfft2(xn[0].astype(np.float64), axes=(0, 1), norm="ortho")  # [H, WF, C]
    mf = np.fft.rfft2(np.asarray(mod_embed[0], np.float64), axes=(0, 1), norm="ortho")
    mr_f = np.ascontiguousarray(mf.real.astype(np.float32)).reshape(SPEC_TOT, EMBED)
    mi_f = np.ascontiguousarray(mf.imag.astype(np.float32)).reshape(SPEC_TOT, EMBED)

    nc = _build_program()

    HM = HID_M // 128
    OM = OUT_M // 128
    HF = HID_F // 128
    OF = OUT_F // 128

    # weights: partition-major packing so every device DMA is contiguous
    w1m_h = _fp8(np.asarray(m_c1_w, np.float32).reshape(HM, 128, KC, 128).transpose(3, 0, 2, 1))
    w2m_h = _fp8(np.asarray(m_c2_w, np.float32).reshape(OM, 128, HM, 128)
                 .transpose(0, 3, 2, 1).reshape(OUT_M, HM, 128))
    w1f_h = _fp8(np.asarray(f_c1_w, np.float32).reshape(HF, 128, KC, 128).transpose(3, 0, 2, 1))
    w2f_pack = (np.asarray(f_c2_w, np.float32).reshape(OF, 128, HF, 128)
                .transpose(0, 3, 2, 1).reshape(OUT_F, HF, 128))
    w2f8_h = _fp8(w2f_pack[:OUT_F // 2])
    w2fb_h = _bf16(w2f_pack[OUT_F // 2:])
    shared = {
        "w1m": w1m_h, "b1m": np.asarray(m_c1_b, np.float32).reshape(HM, 128).T.copy(),
        "w2m": w2m_h, "b2m": np.asarray(m_c2_b, np.float32).reshape(OM, 128).T.copy(),
        "w1f": w1f_h, "b1f": np.asarray(f_c1_b, np.float32).reshape(HF, 128).T.copy(),
        "w2f8": w2f8_h, "w2fb": w2fb_h,
        "b2f": np.asarray(f_c2_b, np.float32).reshape(OF, 128).T.copy(),
    }

    modp = mod_embed[0].reshape(H * W, EMBED)
    in_maps = []
    for k in range(N_CORES):
        m = dict(shared)
        a1 = modp[k * S1:(k + 1) * S1].T.reshape(KC, 128, S1).transpose(1, 0, 2)
        m["a1"] = _fp8(a1)
        cat = np.concatenate(
            [mr_f[k * S2:(k + 1) * S2], mi_f[k * S2:(k + 1) * S2]], 0
        )  # [PXF, EMBED]
        a23 = cat.T.reshape(KC, 128, PXF).transpose(1, 0, 2)
        m["a23"] = _fp8(a23)
        in_maps.append(m)

    res = run_bass_kernel_spmd(nc, in_maps, core_ids=list(range(N_CORES)))
    global LAST_RESULT
    LAST_RESULT = res

    # reassemble (device already applied final ReLU)
    ss_mlp = np.concatenate(
        [res.results[k]["o1"].astype(np.float32).T for k in range(N_CORES)], 0
    )  # [16384, 6144]
    fo = [res.results[k]["o2"].astype(np.float32) for k in range(N_CORES)]
    fo_re = np.concatenate([f[:, :S2].T for f in fo], 0)   # [8320, 1536]
    fo_im = np.concatenate([f[:, S2:].T for f in fo], 0)

    # ---- host: rest of the filter ----
    xr = xf.real.astype(np.float32).reshape(1, H, WF, BLOCKS, BS)
    xi = xf.imag.astype(np.float32).reshape(1, H, WF, BLOCKS, BS)
    w1_ = np.asarray(w1, np.float32)
    b1_ = np.asarray(b1, np.float32)
    w2_ = np.asarray(w2, np.float32)
    b2_ = np.asarray(b2, np.float32)
    o1_re = _blockmm(xr, w1_[0]) - _blockmm(xi, w1_[1]) + b1_[0]
    o1_im = _blockmm(xi, w1_[0]) + _blockmm(xr, w1_[1]) + b1_[1]

    sc_re = 1.0 + fo_re[:, :EMBED].reshape(1, H, WF, BLOCKS, BS)
    sh_re = fo_re[:, EMBED:].reshape(1, H, WF, BLOCKS, BS)
    sc_im = 1.0 + fo_im[:, :EMBED].reshape(1, H, WF, BLOCKS, BS)
    sh_im = fo_im[:, EMBED:].reshape(1, H, WF, BLOCKS, BS)

    n_re = o1_re * sc_re - o1_im * sc_im + sh_re
    n_im = o1_im * sc_re + o1_re * sc_im + sh_im
    o1_re = np.maximum(n_re, 0.0)
    o1_im = np.maximum(n_im, 0.0)

    o2_re = _softshrink(_blockmm(o1_re, w2_[0]) - _blockmm(o1_im, w2_[1]) + b2_[0], LAMBD)
    o2_im = _softshrink(_blockmm(o1_im, w2_[0]) + _blockmm(o1_re, w2_[1]) + b2_[1], LAMBD)

    spec = (o2_re + 1j * o2_im).reshape(H, WF, EMBED)
    filt = np.fft.irfft2(spec, s=(H, W), axes=(0, 1), norm="ortho").astype(np.float32)
    h_mid = filt[None] + xn + residual  # filter bias (xn) + double_skip residual

    # ---- host: second half (device did scale/shift) ----
    h2 = _layernorm(h_mid, np.asarray(norm2_w, np.float32), np.asarray(norm2_b, np.float32))
    scale = 1.0 + ss_mlp[:, :LATENT].reshape(1, H, W, LATENT)
    shift = ss_mlp[:, LATENT:].reshape(1, H, W, LATENT)
    hh = h2.reshape(H * W, EMBED) @ np.asarray(fc1_w, np.float32).T + np.asarray(fc1_b, np.float32)
    hh = hh.reshape(1, H, W, LATENT) * scale + shift
    hh = _gelu(hh)
    out = hh.reshape(H * W, LATENT) @ np.asarray(fc2_w, np.float32).T + np.asarray(fc2_b, np.float32)
    return (out.reshape(1, H, W, EMBED) + h_mid).astype(np.float32)


# revision 21
# speedup vs baseline: 1.0454x; 1.0454x over previous
import sys

sys.path.insert(0, "/opt/trn_rl_repo")
import numpy as np
import ml_dtypes

import concourse.bass as bass
import concourse.tile as tile
import concourse.bacc as bacc
from concourse import mybir
from concourse.bass_utils import run_bass_kernel_spmd

# bass_utils' axon trace path hard-imports antenv.axon_hooks; provide a
# null-hook shim when the image lacks it so tracing degrades gracefully
# instead of crashing kernel().
try:
    import antenv.axon_hooks  # noqa: F401
except ImportError:
    import types as _types

    _hook_store = {"fn": None}
    _m = _types.ModuleType("antenv.axon_hooks")
    _m.set_axon_ntff_profile_hook = lambda h: _hook_store.__setitem__("fn", h)
    _m.get_axon_ntff_profile_hook = lambda: _hook_store["fn"]
    sys.modules["antenv.axon_hooks"] = _m

BF16 = mybir.dt.bfloat16
F32 = mybir.dt.float32
FP8 = mybir.dt.float8e4
DR = mybir.MatmulPerfMode.DoubleRow
RELU = mybir.ActivationFunctionType.Relu

N_CORES = 8
EMBED = 768
KC = 6            # 768 / 128 contraction chunks
BLOCKS = 8
BS = 96
LATENT = 4 * EMBED            # 3072
HID_M = 4 * LATENT            # 12288
OUT_M = 2 * LATENT            # 6144
HID_F = 4 * EMBED             # 3072
OUT_F = 2 * EMBED             # 1536
LAMBD = 0.01
EPS = 1e-5
H = 128
W = 128
WF = 65
SPEC_TOT = H * WF             # 8320
S1 = (H * W) // N_CORES       # 2048 spatial px per core
S2 = SPEC_TOT // N_CORES      # 1040 spectral px per core
PXF = 2 * S2                  # 2080 (re ++ im)
NBF = 5
BLKF = PXF // NBF             # 416 (psum-bank safe)


def _erf(x):
    a1, a2, a3, a4, a5, p = (
        0.254829592, -0.284496736, 1.421413741, -1.453152027, 1.061405429, 0.3275911,
    )
    s = np.sign(x)
    ax = np.abs(x)
    t = 1.0 / (1.0 + p * ax)
    y = 1.0 - (((((a5 * t + a4) * t) + a3) * t + a2) * t + a1) * t * np.exp(-ax * ax)
    return s * y


def _gelu(x):
    return 0.5 * x * (1.0 + _erf(x / np.sqrt(2.0)))


def _layernorm(x, w, b):
    m = x.mean(-1, keepdims=True)
    v = x.var(-1, keepdims=True)
    return (x - m) / np.sqrt(v + EPS) * w + b


def _softshrink(x, l):
    return np.where(x > l, x - l, np.where(x < -l, x + l, 0.0)).astype(np.float32)


def _blockmm(x, w):
    return np.einsum("nyxbi,bio->nyxbo", x, w, optimize=True)


_PROGRAM = None
LAST_RESULT = None


def _build_program():
    global _PROGRAM
    if _PROGRAM is not None:
        return _PROGRAM
    from contextlib import ExitStack

    nc = bacc.Bacc("TRN2", target_bir_lowering=False, debug=False, num_devices=N_CORES)

    A1 = nc.dram_tensor("a1", [128, KC, S1], FP8, kind="ExternalInput")
    W1M = nc.dram_tensor("w1m", [128, HID_M // 128, KC, 128], FP8, kind="ExternalInput")
    W2M = nc.dram_tensor("w2m", [OUT_M, HID_M // 128, 128], FP8, kind="ExternalInput")
    B1M = nc.dram_tensor("b1m", [128, HID_M // 128], F32, kind="ExternalInput")
    B2M = nc.dram_tensor("b2m", [128, OUT_M // 128], F32, kind="ExternalInput")
    A23 = nc.dram_tensor("a23", [128, KC, PXF], FP8, kind="ExternalInput")
    W1F = nc.dram_tensor("w1f", [128, HID_F // 128, KC, 128], FP8, kind="ExternalInput")
    W2F8 = nc.dram_tensor("w2f8", [OUT_F // 2, HID_F // 128, 128], FP8, kind="ExternalInput")
    W2FB = nc.dram_tensor("w2fb", [OUT_F // 2, HID_F // 128, 128], BF16, kind="ExternalInput")
    B1F = nc.dram_tensor("b1f", [128, HID_F // 128], F32, kind="ExternalInput")
    B2F = nc.dram_tensor("b2f", [128, OUT_F // 128], F32, kind="ExternalInput")

    O1 = nc.dram_tensor("o1", [OUT_M, S1], BF16, kind="ExternalOutput")
    O2 = nc.dram_tensor("o2", [OUT_F, PXF], BF16, kind="ExternalOutput")

    HM = HID_M // 128   # 96
    OM = OUT_M // 128   # 48
    HF = HID_F // 128   # 24
    OF = OUT_F // 128   # 12

    with tile.TileContext(nc) as tc, ExitStack() as octx:
        # ---------- M pipeline: fp8 DoubleRow, 2 pixel halves of 1024 ----------
        with ExitStack() as mctx:
            cp = mctx.enter_context(tc.tile_pool(name="m_const", bufs=1))
            w1p = mctx.enter_context(tc.tile_pool(name="m_w1", bufs=2))
            w2p = mctx.enter_context(tc.tile_pool(name="m_w2", bufs=2))
            h1p = mctx.enter_context(tc.tile_pool(name="m_h1", bufs=1))
            op = mctx.enter_context(tc.tile_pool(name="m_out", bufs=4))
            pp = mctx.enter_context(tc.tile_pool(name="m_ps", bufs=8, space="PSUM"))

            # a1 split per pixel-half so the first conv1 matmul only waits
            # for half the activation DMA.
            a1h = []
            for hf in range(2):
                t = cp.tile([128, KC, 1024], FP8, name=f"a1h{hf}")
                nc.sync.dma_start(t[:], A1[:, :, bass.ds(hf * 1024, 1024)])
                a1h.append(t)
            b1t = cp.tile([128, HM], F32)
            nc.sync.dma_start(b1t[:], B1M[:])
            b2t = cp.tile([128, OM], F32)
            nc.sync.dma_start(b2t[:], B2M[:])

            for hf in range(2):
                h1t = h1p.tile([128, HM, 1024], FP8, tag="h1")
                # conv1: 96 hid strips in groups of 8
                for g in range(12):
                    w1t = w1p.tile([128, 8, KC, 128], FP8, tag="w1")
                    nc.sync.dma_start(w1t[:], W1M[:, bass.ds(g * 8, 8), :, :])
                    for s in range(8):
                        i = g * 8 + s
                        for sb in range(2):
                            ps = pp.tile([128, 512], F32, tag="ps")
                            for c in range(3):
                                nc.tensor.matmul(
                                    ps[:],
                                    w1t[:, s, bass.ds(2 * c, 2), :],
                                    a1h[hf][:, bass.ds(2 * c, 2), bass.ds(sb * 512, 512)],
                                    start=(c == 0), stop=(c == 2),
                                    perf_mode=DR,
                                )
                            nc.scalar.activation(
                                h1t[:, i, bass.ds(sb * 512, 512)], ps[:], RELU,
                                bias=b1t[:, i:i + 1],
                            )
                # conv2: 48 out strips, stream weights
                for o in range(OM):
                    w2t = w2p.tile([128, HM, 128], FP8, tag="w2")
                    nc.sync.dma_start(w2t[:], W2M[bass.ds(o * 128, 128), :, :])
                    for sb in range(2):
                        ps = pp.tile([128, 512], F32, tag="ps")
                        for j in range(48):
                            nc.tensor.matmul(
                                ps[:],
                                w2t[:, bass.ds(2 * j, 2), :],
                                h1t[:, bass.ds(2 * j, 2), bass.ds(sb * 512, 512)],
                                start=(j == 0), stop=(j == 47),
                                perf_mode=DR,
                            )
                        ot = op.tile([128, 512], BF16, tag="ot")
                        nc.scalar.activation(ot[:], ps[:], RELU, bias=b2t[:, o:o + 1])
                        nc.sync.dma_start(
                            O1[bass.ds(o * 128, 128), bass.ds(hf * 1024 + sb * 512, 512)],
                            ot[:],
                        )

        # ---------- F pipeline: fp8 conv1 (DoubleRow) + bf16 conv2 ----------
        with ExitStack() as fctx:
            cfp = fctx.enter_context(tc.tile_pool(name="f_const", bufs=1))
            w1fp = fctx.enter_context(tc.tile_pool(name="f_w1", bufs=2))
            w2fp = fctx.enter_context(tc.tile_pool(name="f_w2", bufs=2))
            h1fp = fctx.enter_context(tc.tile_pool(name="f_h1", bufs=1))
            ofp = fctx.enter_context(tc.tile_pool(name="f_out", bufs=4))
            fpp = fctx.enter_context(tc.tile_pool(name="f_ps", bufs=8, space="PSUM"))

            # a23 chunked per pixel-block so the first conv1 group waits
            # only for chunk 0 (+ first w1f strip group), not the full
            # F-input DMA — shrinks the M->F PE bubble.
            a23c = []
            for nb in range(NBF):
                t = cfp.tile([128, KC, BLKF], FP8, name=f"a23c{nb}")
                nc.sync.dma_start(t[:], A23[:, :, bass.ds(nb * BLKF, BLKF)])
                a23c.append(t)
            fb1t = cfp.tile([128, HF], F32)
            nc.sync.dma_start(fb1t[:], B1F[:])
            fb2t = cfp.tile([128, OF], F32)
            nc.sync.dma_start(fb2t[:], B2F[:])

            h1ft = h1fp.tile([128, HF, PXF], BF16)
            h1f8t = h1fp.tile([128, HF, PXF], FP8)
            for g in range(HF // 8):
                w1gt = w1fp.tile([128, 8, KC, 128], FP8, tag="w1f")
                nc.sync.dma_start(w1gt[:], W1F[:, bass.ds(g * 8, 8), :, :])
                for s in range(8):
                    i = g * 8 + s
                    for nb in range(NBF):
                        ps = fpp.tile([128, BLKF], F32, tag="ps")
                        for c in range(KC // 2):
                            nc.tensor.matmul(
                                ps[:],
                                w1gt[:, s, bass.ds(2 * c, 2), :],
                                a23c[nb][:, bass.ds(2 * c, 2), :],
                                start=(c == 0), stop=(c == KC // 2 - 1),
                                perf_mode=DR,
                            )
                        nc.scalar.activation(
                            h1ft[:, i, bass.ds(nb * BLKF, BLKF)], ps[:], RELU,
                            bias=fb1t[:, i:i + 1],
                        )
                        nc.vector.tensor_copy(
                            h1f8t[:, i, bass.ds(nb * BLKF, BLKF)],
                            h1ft[:, i, bass.ds(nb * BLKF, BLKF)],
                        )
            # scale half (output rows 0:768): fp8 DoubleRow — the scale
            # multiplies the small-amplitude spectral signal, so its fp8
            # noise is strongly attenuated; shift half stays bf16.
            for o in range(OF // 2):
                w2ft = w2fp.tile([128, HF, 128], FP8, tag="w2f8")
                nc.sync.dma_start(w2ft[:], W2F8[bass.ds(o * 128, 128), :, :])
                for nb in range(NBF):
                    ps = fpp.tile([128, BLKF], F32, tag="ps")
                    for j in range(HF // 2):
                        nc.tensor.matmul(
                            ps[:],
                            w2ft[:, bass.ds(2 * j, 2), :],
                            h1f8t[:, bass.ds(2 * j, 2), bass.ds(nb * BLKF, BLKF)],
                            start=(j == 0), stop=(j == HF // 2 - 1),
                            perf_mode=DR,
                        )
                    ot = ofp.tile([128, BLKF], BF16, tag="otf")
                    nc.scalar.activation(ot[:], ps[:], RELU, bias=fb2t[:, o:o + 1])
                    nc.sync.dma_start(
                        O2[bass.ds(o * 128, 128), bass.ds(nb * BLKF, BLKF)], ot[:]
                    )
            for oo in range(OF // 2):
                o = OF // 2 + oo
                w2ft = w2fp.tile([128, HF, 128], BF16, tag="w2fb")
                nc.sync.dma_start(w2ft[:], W2FB[bass.ds(oo * 128, 128), :, :])
                for nb in range(NBF):
                    ps = fpp.tile([128, BLKF], F32, tag="ps")
                    for j in range(HF):
                        nc.tensor.matmul(
                            ps[:],
                            w2ft[:, j, :],
                            h1ft[:, j, bass.ds(nb * BLKF, BLKF)],
                            start=(j == 0), stop=(j == HF - 1),
                        )
                    ot = ofp.tile([128, BLKF], BF16, tag="otf")
                    nc.scalar.activation(ot[:], ps[:], RELU, bias=fb2t[:, o:o + 1])
                    nc.sync.dma_start(
                        O2[bass.ds(o * 128, 128), bass.ds(nb * BLKF, BLKF)], ot[:]
                    )

    nc.compile()
    _PROGRAM = nc
    return nc


def _fp8(x):
    return np.clip(np.ascontiguousarray(x), -240, 240).astype(ml_dtypes.float8_e4m3)


def _bf16(x):
    return np.ascontiguousarray(x).astype(ml_dtypes.bfloat16)


def kernel(x, mod_embed, norm1_w, norm1_b, norm2_w, norm2_b, w1, b1, w2, b2,
           f_c1_w, f_c1_b, f_c2_w, f_c2_b, fc1_w, fc1_b, fc2_w, fc2_b,
           m_c1_w, m_c1_b, m_c2_w, m_c2_b):
    x = np.asarray(x, np.float32)
    mod_embed = np.asarray(mod_embed, np.float32)
    B = x.shape[0]
    assert B == 1 and x.shape == (1, H, W, EMBED)

    # ---- host: LN1 + forward FFTs (cheap) ----
    residual = x
    xn = _layernorm(x, np.asarray(norm1_w, np.float32), np.asarray(norm1_b, np.float32))
    xf = np.fft.rfft2(xn[0].astype(np.float64), axes=(0, 1), norm="ortho")  # [H, WF, C]
    mf = np.fft.rfft2(np.asarray(mod_embed[0], np.float64), axes=(0, 1), norm="ortho")
    mr_f = np.ascontiguousarray(mf.real.astype(np.float32)).reshape(SPEC_TOT, EMBED)
    mi_f = np.ascontiguousarray(mf.imag.astype(np.float32)).reshape(SPEC_TOT, EMBED)

    nc = _build_program()

    HM = HID_M // 128
    OM = OUT_M // 128
    HF = HID_F // 128
    OF = OUT_F // 128

    # weights: partition-major packing so every device DMA is contiguous
    w1m_h = _fp8(np.asarray(m_c1_w, np.float32).reshape(HM, 128, KC, 128).transpose(3, 0, 2, 1))
    w2m_h = _fp8(np.asarray(m_c2_w, np.float32).reshape(OM, 128, HM, 128)
                 .transpose(0, 3, 2, 1).reshape(OUT_M, HM, 128))
    w1f_h = _fp8(np.asarray(f_c1_w, np.float32).reshape(HF, 128, KC, 128).transpose(3, 0, 2, 1))
    w2f_pack = (np.asarray(f_c2_w, np.float32).reshape(OF, 128, HF, 128)
                .transpose(0, 3, 2, 1).reshape(OUT_F, HF, 128))
    w2f8_h = _fp8(w2f_pack[:OUT_F // 2])
    w2fb_h = _bf16(w2f_pack[OUT_F // 2:])
    shared = {
        "w1m": w1m_h, "b1m": np.asarray(m_c1_b, np.float32).reshape(HM, 128).T.copy(),
        "w2m": w2m_h, "b2m": np.asarray(m_c2_b, np.float32).reshape(OM, 128).T.copy(),
        "w1f": w1f_h, "b1f": np.asarray(f_c1_b, np.float32).reshape(HF, 128).T.copy(),
        "w2f8": w2f8_h, "w2fb": w2fb_h,
        "b2f": np.asarray(f_c2_b, np.float32).reshape(OF, 128).T.copy(),
    }

    modp = mod_embed[0].reshape(H * W, EMBED)
    in_maps = []
    for k in range(N_CORES):
        m = dict(shared)
        a1 = modp[k * S1:(k + 1) * S1].T.reshape(KC, 128, S1).transpose(1, 0, 2)
        m["a1"] = _fp8(a1)
        cat = np.concatenate(
            [mr_f[k * S2:(k + 1) * S2], mi_f[k * S2:(k + 1) * S2]], 0
        )  # [PXF, EMBED]
        a23 = cat.T.reshape(KC, 128, PXF).transpose(1, 0, 2)
        m["a23"] = _fp8(a23)
        in_maps.append(m)

    res = run_bass_kernel_spmd(nc, in_maps, core_ids=list(range(N_CORES)))
    global LAST_RESULT
    LAST_RESULT = res

    # reassemble (device already applied final ReLU)
    ss_mlp = np.concatenate(
        [res.results[k]["o1"].astype(np.float32).T for k in range(N_CORES)], 0
    )  # [16384, 6144]
    fo = [res.results[k]["o2"].astype(np.float32) for k in range(N_CORES)]
    fo_re = np.concatenate([f[:, :S2].T for f in fo], 0)   # [8320, 1536]
    fo_im = np.concatenate([f[:, S2:].T for f in fo], 0)

    # ---- host: rest of the filter ----
    xr = xf.real.astype(np.float32).reshape(1, H, WF, BLOCKS, BS)
    xi = xf.imag.astype(np.float32).reshape(1, H, WF, BLOCKS, BS)
    w1_ = np.asarray(w1, np.float32)
    b1_ = np.asarray(b1, np.float32)
    w2_ = np.asarray(w2, np.float32)
    b2_ = np.asarray(b2, np.float32)
    o1_re = _blockmm(xr, w1_[0]) - _blockmm(xi, w1_[1]) + b1_[0]
    o1_im = _blockmm(xi, w1_[0]) + _blockmm(xr, w1_[1]) + b1_[1]

    sc_re = 1.0 + fo_re[:, :EMBED].reshape(1, H, WF, BLOCKS, BS)
    sh_re = fo_re[:, EMBED:].reshape(1, H, WF, BLOCKS, BS)
    sc_im = 1.0 + fo_im[:, :EMBED].reshape(1, H, WF, BLOCKS, BS)
    sh_im = fo_im[:, EMBED:].reshape(1, H, WF, BLOCKS, BS)

    n_re = o1_re * sc_re - o1_im * sc_im + sh_re
    n_im = o1_im * sc_re + o1_re * sc_im + sh_im
    o1_re = np.maximum(n_re, 0.0)
    o1_im = np.maximum(n_im, 0.0)

    o2_re = _softshrink(_blockmm(o1_re, w2_[0]) - _blockmm(o1_im, w2_[1]) + b2_[0], LAMBD)
    o2_im = _softshrink(_blockmm(o1_im, w2_[0]) + _blockmm(o1_re, w2_[1]) + b2_[1], LAMBD)

    spec = (o2_re + 1j * o2_im).reshape(H, WF, EMBED)
    filt = np.fft.irfft2(spec, s=(H, W), axes=(0, 1), norm="ortho").astype(np.float32)
    h_mid = filt[None] + xn + residual  # filter bias (xn) + double_skip residual

    # ---- host: second half (device did scale/shift) ----
    h2 = _layernorm(h_mid, np.asarray(norm2_w, np.float32), np.asarray(norm2_b, np.float32))
    scale = 1.0 + ss_mlp[:, :LATENT].reshape(1, H, W, LATENT)
    shift = ss_mlp[:, LATENT:].reshape(1, H, W, LATENT)
    hh = h2.reshape(H * W, EMBED) @ np.asarray(fc1_w, np.float32).T + np.asarray(fc1_b, np.float32)
    hh = hh.reshape(1, H, W, LATENT) * scale + shift
    hh = _gelu(hh)
    out = hh.reshape(H * W, LATENT) @ np.asarray(fc2_w, np.float32).T + np.asarray(fc2_b, np.float32)
    return (out.reshape(1, H, W, EMBED) + h_mid).astype(np.float32)


# revision 24
# speedup vs baseline: 1.0514x; 1.0057x over previous
import sys

sys.path.insert(0, "/opt/trn_rl_repo")
import numpy as np
import ml_dtypes

import concourse.bass as bass
import concourse.tile as tile
import concourse.bacc as bacc
from concourse import mybir
from concourse.bass_utils import run_bass_kernel_spmd

# bass_utils' axon trace path hard-imports antenv.axon_hooks; provide a
# null-hook shim when the image lacks it so tracing degrades gracefully
# instead of crashing kernel().
try:
    import antenv.axon_hooks  # noqa: F401
except ImportError:
    import types as _types

    _hook_store = {"fn": None}
    _m = _types.ModuleType("antenv.axon_hooks")
    _m.set_axon_ntff_profile_hook = lambda h: _hook_store.__setitem__("fn", h)
    _m.get_axon_ntff_profile_hook = lambda: _hook_store["fn"]
    sys.modules["antenv.axon_hooks"] = _m

BF16 = mybir.dt.bfloat16
F32 = mybir.dt.float32
FP8 = mybir.dt.float8e4
DR = mybir.MatmulPerfMode.DoubleRow
RELU = mybir.ActivationFunctionType.Relu

N_CORES = 8
EMBED = 768
KC = 6            # 768 / 128 contraction chunks
BLOCKS = 8
BS = 96
LATENT = 4 * EMBED            # 3072
HID_M = 4 * LATENT            # 12288
OUT_M = 2 * LATENT            # 6144
HID_F = 4 * EMBED             # 3072
OUT_F = 2 * EMBED             # 1536
LAMBD = 0.01
EPS = 1e-5
H = 128
W = 128
WF = 65
SPEC_TOT = H * WF             # 8320
S1 = (H * W) // N_CORES       # 2048 spatial px per core
S2 = SPEC_TOT // N_CORES      # 1040 spectral px per core
PXF = 2 * S2                  # 2080 (re ++ im)
NBF = 5
BLKF = PXF // NBF             # 416 (psum-bank safe)


def _erf(x):
    a1, a2, a3, a4, a5, p = (
        0.254829592, -0.284496736, 1.421413741, -1.453152027, 1.061405429, 0.3275911,
    )
    s = np.sign(x)
    ax = np.abs(x)
    t = 1.0 / (1.0 + p * ax)
    y = 1.0 - (((((a5 * t + a4) * t) + a3) * t + a2) * t + a1) * t * np.exp(-ax * ax)
    return s * y


def _gelu(x):
    return 0.5 * x * (1.0 + _erf(x / np.sqrt(2.0)))


def _layernorm(x, w, b):
    m = x.mean(-1, keepdims=True)
    v = x.var(-1, keepdims=True)
    return (x - m) / np.sqrt(v + EPS) * w + b


def _softshrink(x, l):
    return np.where(x > l, x - l, np.where(x < -l, x + l, 0.0)).astype(np.float32)


def _blockmm(x, w):
    return np.einsum("nyxbi,bio->nyxbo", x, w, optimize=True)


_PROGRAM = None
LAST_RESULT = None


def _build_program():
    global _PROGRAM
    if _PROGRAM is not None:
        return _PROGRAM
    from contextlib import ExitStack

    nc = bacc.Bacc("TRN2", target_bir_lowering=False, debug=False, num_devices=N_CORES)

    A1 = nc.dram_tensor("a1", [128, KC, S1], FP8, kind="ExternalInput")
    W1M = nc.dram_tensor("w1m", [128, HID_M // 128, KC, 128], FP8, kind="ExternalInput")
    W2M = nc.dram_tensor("w2m", [OUT_M, HID_M // 128, 128], FP8, kind="ExternalInput")
    B1M = nc.dram_tensor("b1m", [128, HID_M // 128], F32, kind="ExternalInput")
    B2M = nc.dram_tensor("b2m", [128, OUT_M // 128], F32, kind="ExternalInput")
    A23 = nc.dram_tensor("a23", [128, KC, PXF], FP8, kind="ExternalInput")
    W1F = nc.dram_tensor("w1f", [128, HID_F // 128, KC, 128], FP8, kind="ExternalInput")
    W2F8 = nc.dram_tensor("w2f8", [OUT_F // 2, HID_F // 128, 128], FP8, kind="ExternalInput")
    W2FB = nc.dram_tensor("w2fb", [OUT_F // 2, HID_F // 128, 128], BF16, kind="ExternalInput")
    B1F = nc.dram_tensor("b1f", [128, HID_F // 128], F32, kind="ExternalInput")
    B2F = nc.dram_tensor("b2f", [128, OUT_F // 128], F32, kind="ExternalInput")

    O1 = nc.dram_tensor("o1", [OUT_M, S1], BF16, kind="ExternalOutput")
    O2 = nc.dram_tensor("o2", [OUT_F, PXF], BF16, kind="ExternalOutput")

    HM = HID_M // 128   # 96
    OM = OUT_M // 128   # 48
    HF = HID_F // 128   # 24
    OF = OUT_F // 128   # 12

    with tile.TileContext(nc) as tc, ExitStack() as octx:
        # F-phase constants live in a bottom pool padded to exactly 32 KB/
        # partition (keeps the M pools at a power-of-2 base) and are DMA'd
        # during M conv1 so the M->F transition has no input-DMA bubble.
        cfp = octx.enter_context(tc.tile_pool(name="f_const", bufs=1))
        a23c = []
        for nb in range(NBF):
            a23c.append(cfp.tile([128, KC, BLKF], FP8, name=f"a23c{nb}"))
        w1ft = cfp.tile([128, HF, KC, 128], FP8)
        fb1t = cfp.tile([128, HF], F32)
        fb2t = cfp.tile([128, OF], F32)
        cfp.tile([128, 1712], FP8, name="pad")  # pad pool to 32768 B/part

        def _issue_f_const_dmas():
            for nb in range(NBF):
                nc.sync.dma_start(a23c[nb][:], A23[:, :, bass.ds(nb * BLKF, BLKF)])
            nc.sync.dma_start(w1ft[:], W1F[:])
            nc.sync.dma_start(fb1t[:], B1F[:])
            nc.sync.dma_start(fb2t[:], B2F[:])

        # ---------- M pipeline: fp8 DoubleRow, 2 pixel halves of 1024 ----------
        with ExitStack() as mctx:
            cp = mctx.enter_context(tc.tile_pool(name="m_const", bufs=1))
            w1p = mctx.enter_context(tc.tile_pool(name="m_w1", bufs=2))
            w2p = mctx.enter_context(tc.tile_pool(name="m_w2", bufs=2))
            h1p = mctx.enter_context(tc.tile_pool(name="m_h1", bufs=1))
            op = mctx.enter_context(tc.tile_pool(name="m_out", bufs=4))
            pp = mctx.enter_context(tc.tile_pool(name="m_ps", bufs=8, space="PSUM"))

            # a1 split per pixel-half so the first conv1 matmul only waits
            # for half the activation DMA.
            a1h = []
            for hf in range(2):
                t = cp.tile([128, KC, 1024], FP8, name=f"a1h{hf}")
                nc.sync.dma_start(t[:], A1[:, :, bass.ds(hf * 1024, 1024)])
                a1h.append(t)
            b1t = cp.tile([128, HM], F32)
            nc.sync.dma_start(b1t[:], B1M[:])
            b2t = cp.tile([128, OM], F32)
            nc.sync.dma_start(b2t[:], B2M[:])

            for hf in range(2):
                h1t = h1p.tile([128, HM, 1024], FP8, tag="h1")
                # conv1: 96 hid strips in groups of 8
                for g in range(12):
                    w1t = w1p.tile([128, 8, KC, 128], FP8, tag="w1")
                    nc.sync.dma_start(w1t[:], W1M[:, bass.ds(g * 8, 8), :, :])
                    if hf == 0 and g == 1:
                        # queued behind M's first loads: doesn't delay start
                        _issue_f_const_dmas()
                    for s in range(8):
                        i = g * 8 + s
                        for sb in range(2):
                            ps = pp.tile([128, 512], F32, tag="ps")
                            for c in range(3):
                                nc.tensor.matmul(
                                    ps[:],
                                    w1t[:, s, bass.ds(2 * c, 2), :],
                                    a1h[hf][:, bass.ds(2 * c, 2), bass.ds(sb * 512, 512)],
                                    start=(c == 0), stop=(c == 2),
                                    perf_mode=DR,
                                )
                            nc.scalar.activation(
                                h1t[:, i, bass.ds(sb * 512, 512)], ps[:], RELU,
                                bias=b1t[:, i:i + 1],
                            )
                # conv2: 48 out strips, stream weights
                for o in range(OM):
                    w2t = w2p.tile([128, HM, 128], FP8, tag="w2")
                    nc.sync.dma_start(w2t[:], W2M[bass.ds(o * 128, 128), :, :])
                    for sb in range(2):
                        ps = pp.tile([128, 512], F32, tag="ps")
                        for j in range(48):
                            nc.tensor.matmul(
                                ps[:],
                                w2t[:, bass.ds(2 * j, 2), :],
                                h1t[:, bass.ds(2 * j, 2), bass.ds(sb * 512, 512)],
                                start=(j == 0), stop=(j == 47),
                                perf_mode=DR,
                            )
                        ot = op.tile([128, 512], BF16, tag="ot")
                        nc.scalar.activation(ot[:], ps[:], RELU, bias=b2t[:, o:o + 1])
                        nc.sync.dma_start(
                            O1[bass.ds(o * 128, 128), bass.ds(hf * 1024 + sb * 512, 512)],
                            ot[:],
                        )

        # ---------- F pipeline: fp8 conv1 (DoubleRow) + bf16 conv2 ----------
        with ExitStack() as fctx:
            w2fp = fctx.enter_context(tc.tile_pool(name="f_w2", bufs=2))
            h1fp = fctx.enter_context(tc.tile_pool(name="f_h1", bufs=1))
            ofp = fctx.enter_context(tc.tile_pool(name="f_out", bufs=4))
            fpp = fctx.enter_context(tc.tile_pool(name="f_ps", bufs=8, space="PSUM"))

            h1ft = h1fp.tile([128, HF, PXF], BF16)
            h1f8t = h1fp.tile([128, HF, PXF], FP8)
            for i in range(HF):
                for nb in range(NBF):
                    ps = fpp.tile([128, BLKF], F32, tag="ps")
                    for c in range(KC // 2):
                        nc.tensor.matmul(
                            ps[:],
                            w1ft[:, i, bass.ds(2 * c, 2), :],
                            a23c[nb][:, bass.ds(2 * c, 2), :],
                            start=(c == 0), stop=(c == KC // 2 - 1),
                            perf_mode=DR,
                        )
                    nc.scalar.activation(
                        h1ft[:, i, bass.ds(nb * BLKF, BLKF)], ps[:], RELU,
                        bias=fb1t[:, i:i + 1],
                    )
                    nc.vector.tensor_copy(
                        h1f8t[:, i, bass.ds(nb * BLKF, BLKF)],
                        h1ft[:, i, bass.ds(nb * BLKF, BLKF)],
                    )
            # scale half (output rows 0:768): fp8 DoubleRow — the scale
            # multiplies the small-amplitude spectral signal, so its fp8
            # noise is strongly attenuated; shift half stays bf16.
            for o in range(OF // 2):
                w2ft = w2fp.tile([128, HF, 128], FP8, tag="w2f8")
                nc.sync.dma_start(w2ft[:], W2F8[bass.ds(o * 128, 128), :, :])
                for nb in range(NBF):
                    ps = fpp.tile([128, BLKF], F32, tag="ps")
                    for j in range(HF // 2):
                        nc.tensor.matmul(
                            ps[:],
                            w2ft[:, bass.ds(2 * j, 2), :],
                            h1f8t[:, bass.ds(2 * j, 2), bass.ds(nb * BLKF, BLKF)],
                            start=(j == 0), stop=(j == HF // 2 - 1),
                            perf_mode=DR,
                        )
                    ot = ofp.tile([128, BLKF], BF16, tag="otf")
                    nc.scalar.activation(ot[:], ps[:], RELU, bias=fb2t[:, o:o + 1])
                    nc.sync.dma_start(
                        O2[bass.ds(o * 128, 128), bass.ds(nb * BLKF, BLKF)], ot[:]
                    )
            for oo in range(OF // 2):
                o = OF // 2 + oo
                w2ft = w2fp.tile([128, HF, 128], BF16, tag="w2fb")
                nc.sync.dma_start(w2ft[:], W2FB[bass.ds(oo * 128, 128), :, :])
                for nb in range(NBF):
                    ps = fpp.tile([128, BLKF], F32, tag="ps")
                    for j in range(HF):
                        nc.tensor.matmul(
                            ps[:],
                            w2ft[:, j, :],
                            h1ft[:, j, bass.ds(nb * BLKF, BLKF)],
                            start=(j == 0), stop=(j == HF - 1),
                        )
                    ot = ofp.tile([128, BLKF], BF16, tag="otf")
                    nc.scalar.activation(ot[:], ps[:], RELU, bias=fb2t[:, o:o + 1])
                    nc.sync.dma_start(
                        O2[bass.ds(o * 128, 128), bass.ds(nb * BLKF, BLKF)], ot[:]
                    )

    nc.compile()
    _PROGRAM = nc
    return nc


def _fp8(x):
    return np.clip(np.ascontiguousarray(x), -240, 240).astype(ml_dtypes.float8_e4m3)


def _bf16(x):
    return np.ascontiguousarray(x).astype(ml_dtypes.bfloat16)


def kernel(x, mod_embed, norm1_w, norm1_b, norm2_w, norm2_b, w1, b1, w2, b2,
           f_c1_w, f_c1_b, f_c2_w, f_c2_b, fc1_w, fc1_b, fc2_w, fc2_b,
           m_c1_w, m_c1_b, m_c2_w, m_c2_b):
    x = np.asarray(x, np.float32)
    mod_embed = np.asarray(mod_embed, np.float32)
    B = x.shape[0]
    assert B == 1 and x.shape == (1, H, W, EMBED)

    # ---- host: LN1 + forward FFTs (cheap) ----
    residual = x
    xn = _layernorm(x, np.asarray(norm1_w, np.float32), np.asarray(norm1_b, np.float32))
    xf = np.fft.rfft2(xn[0].astype(np.float64), axes=(0, 1), norm="ortho")  # [H, WF, C]
    mf = np.fft.rfft2(np.asarray(mod_embed[0], np.float64), axes=(0, 1), norm="ortho")
    mr_f = np.ascontiguousarray(mf.real.astype(np.float32)).reshape(SPEC_TOT, EMBED)
    mi_f = np.ascontiguousarray(mf.imag.astype(np.float32)).reshape(SPEC_TOT, EMBED)

    nc = _build_program()

    HM = HID_M // 128
    OM = OUT_M // 128
    HF = HID_F // 128
    OF = OUT_F // 128

    # weights: partition-major packing so every device DMA is contiguous
    w1m_h = _fp8(np.asarray(m_c1_w, np.float32).reshape(HM, 128, KC, 128).transpose(3, 0, 2, 1))
    w2m_h = _fp8(np.asarray(m_c2_w, np.float32).reshape(OM, 128, HM, 128)
                 .transpose(0, 3, 2, 1).reshape(OUT_M, HM, 128))
    w1f_h = _fp8(np.asarray(f_c1_w, np.float32).reshape(HF, 128, KC, 128).transpose(3, 0, 2, 1))
    w2f_pack = (np.asarray(f_c2_w, np.float32).reshape(OF, 128, HF, 128)
                .transpose(0, 3, 2, 1).reshape(OUT_F, HF, 128))
    w2f8_h = _fp8(w2f_pack[:OUT_F // 2])
    w2fb_h = _bf16(w2f_pack[OUT_F // 2:])
    shared = {
        "w1m": w1m_h, "b1m": np.asarray(m_c1_b, np.float32).reshape(HM, 128).T.copy(),
        "w2m": w2m_h, "b2m": np.asarray(m_c2_b, np.float32).reshape(OM, 128).T.copy(),
        "w1f": w1f_h, "b1f": np.asarray(f_c1_b, np.float32).reshape(HF, 128).T.copy(),
        "w2f8": w2f8_h, "w2fb": w2fb_h,
        "b2f": np.asarray(f_c2_b, np.float32).reshape(OF, 128).T.copy(),
    }

    modp = mod_embed[0].reshape(H * W, EMBED)
    in_maps = []
    for k in range(N_CORES):
        m = dict(shared)
        a1 = modp[k * S1:(k + 1) * S1].T.reshape(KC, 128, S1).transpose(1, 0, 2)
        m["a1"] = _fp8(a1)
        cat = np.concatenate(
            [mr_f[k * S2:(k + 1) * S2], mi_f[k * S2:(k + 1) * S2]], 0
        )  # [PXF, EMBED]
        a23 = cat.T.reshape(KC, 128, PXF).transpose(1, 0, 2)
        m["a23"] = _fp8(a23)
        in_maps.append(m)

    res = run_bass_kernel_spmd(nc, in_maps, core_ids=list(range(N_CORES)))
    global LAST_RESULT
    LAST_RESULT = res

    # reassemble (device already applied final ReLU)
    ss_mlp = np.concatenate(
        [res.results[k]["o1"].astype(np.float32).T for k in range(N_CORES)], 0
    )  # [16384, 6144]
    fo = [res.results[k]["o2"].astype(np.float32) for k in range(N_CORES)]
    fo_re = np.concatenate([f[:, :S2].T for f in fo], 0)   # [8320, 1536]
    fo_im = np.concatenate([f[:, S2:].T for f in fo], 0)

    # ---- host: rest of the filter ----
    xr = xf.real.astype(np.float32).reshape(1, H, WF, BLOCKS, BS)
    xi = xf.imag.astype(np.float32).reshape(1, H, WF, BLOCKS, BS)
    w1_ = np.asarray(w1, np.float32)
    b1_ = np.asarray(b1, np.float32)
    w2_ = np.asarray(w2, np.float32)
    b2_ = np.asarray(b2, np.float32)
    o1_re = _blockmm(xr, w1_[0]) - _blockmm(xi, w1_[1]) + b1_[0]
    o1_im = _blockmm(xi, w1_[0]) + _blockmm(xr, w1_[1]) + b1_[1]

    sc_re = 1.0 + fo_re[:, :EMBED].reshape(1, H, WF, BLOCKS, BS)
    sh_re = fo_re[:, EMBED:].reshape(1, H, WF, BLOCKS, BS)
    sc_im = 1.0 + fo_im[:, :EMBED].reshape(1, H, WF, BLOCKS, BS)
    sh_im = fo_im[:, EMBED:].reshape(1, H, WF, BLOCKS, BS)

    n_re = o1_re * sc_re - o1_im * sc_im + sh_re
    n_im = o1_im * sc_re + o1_re * sc_im + sh_im
    o1_re = np.maximum(n_re, 0.0)
    o1_im = np.maximum(n_im, 0.0)

    o2_re = _softshrink(_blockmm(o1_re, w2_[0]) - _blockmm(o1_im, w2_[1]) + b2_[0], LAMBD)
    o2_im = _softshrink(_blockmm(o1_im, w2_[0]) + _blockmm(o1_re, w2_[1]) + b2_[1], LAMBD)

    spec = (o2_re + 1j * o2_im).reshape(H, WF, EMBED)
    filt = np.fft.irfft2(spec, s=(H, W), axes=(0, 1), norm="ortho").astype(np.float32)
    h_mid = filt[None] + xn + residual  # filter bias (xn) + double_skip residual

    # ---- host: second half (device did scale/shift) ----
    h2 = _layernorm(h_mid, np.asarray(norm2_w, np.float32), np.asarray(norm2_b, np.float32))
    scale = 1.0 + ss_mlp[:, :LATENT].reshape(1, H, W, LATENT)
    shift = ss_mlp[:, LATENT:].reshape(1, H, W, LATENT)
    hh = h2.reshape(H * W, EMBED) @ np.asarray(fc1_w, np.float32).T + np.asarray(fc1_b, np.float32)
    hh = hh.reshape(1, H, W, LATENT) * scale + shift
    hh = _gelu(hh)
    out = hh.reshape(H * W, LATENT) @ np.asarray(fc2_w, np.float32).T + np.asarray(fc2_b, np.float32)
    return (out.reshape(1, H, W, EMBED) + h_mid).astype(np.float32)
